# revision 7
# baseline (speedup 1.0000x reference)
"""Trainium2 Bass kernel for ContactMapPredictor.

Computes, for B=2, N1=500, N2=800, D=128:
    p1 = h1 @ W1[:D] ; p2 = h2 @ W1[D:]
    hidden[b,n,m,:] = relu(p1[b,n,:] + p2[b,m,:] + b1)
    pred[b,n,m]     = hidden[b,n,m,:] @ W2 + b2
    mask[b,n,m]     = (S1[b,n]!=0) * (S2[b,m]!=0)
    y[b,n,m]        = (contact_map[b,n,m] < 0.5) * mask[b,n,m]
Returns (pred, y, mask) each reshaped [B, N1*N2].

Strategy: shard N1 across 8 cores (63 rows/core, padded to 504). Per core,
keep D=128 on partitions; for each output row r=(b,n) produce
hid = relu(p2T + p1col) in one fused DVE/ACT op over [128, 800], then reduce
over partitions with PE matmuls (lhsT = W2 placed in column r%32 of a
[128,32] zero-padded variant so 32 rows share a PSUM partition group
accumulatively).
"""

import numpy as np
import ml_dtypes

import bass_rust
import concourse.bass as bass
import concourse.tile as tile
import concourse.mybir as mybir
from concourse.bass_utils import run_bass_kernel_spmd

BF16NP = ml_dtypes.bfloat16
F32 = mybir.dt.float32
BF16 = mybir.dt.bfloat16

B, N1, N2, D = 2, 500, 800, 128
NCORES = 8
N1S = 63              # rows of N1 per core (8*63 = 504 >= 500)
N1P = NCORES * N1S    # padded N1
ROWS = B * N1S        # output rows per core
TH = 0.5
CHUNKS = ((0, 512), (512, 800))  # psum-bank-sized free-dim chunks


def _split_waits(nc):
    """This container's walrus build accepts at most ONE sync-wait command
    per instruction (any extra raises 'Too many sync wait commands' in
    codegen). Tile routinely attaches 2-3 waits to an instruction. Hoist
    all but the last wait onto same-engine NoOp carriers placed directly
    before the instruction — same-sequencer program order preserves the
    happens-before semantics exactly."""
    for blk in nc.m.functions[0].blocks:
        new = []
        for inst in blk.instructions:
            si = inst.sync_info
            waits = list(si.on_wait) if si and si.on_wait else []
            if len(waits) > 1 and inst.engine != mybir.EngineType.Unassigned:
                for w in waits[:-1]:
                    nop = mybir.InstNoOp(
                        name=nc.get_next_instruction_name(), engine=inst.engine
                    )
                    nop.sync_info = bass_rust.SyncInfo(on_wait=[w], on_update=[])
                    new.append(nop)
                si.on_wait = waits[-1:]
                inst.sync_info = si
            new.append(inst)
        blk.instructions = new


def build_nc():
    nc = bass.Bass("TRN2", target_bir_lowering=False, debug=False)

    h1t_d = nc.declare_dram_parameter("h1t", [B, D, N1S], BF16, isOutput=False)
    h2t_d = nc.declare_dram_parameter("h2t", [B, D, N2], BF16, isOutput=False)
    w1a_d = nc.declare_dram_parameter("w1a", [D, D], BF16, isOutput=False)
    w1b_d = nc.declare_dram_parameter("w1b", [D, D], BF16, isOutput=False)
    w2g_d = nc.declare_dram_parameter("w2g", [D, 32 * 32], BF16, isOutput=False)
    b1c_d = nc.declare_dram_parameter("b1c", [D, 1], F32, isOutput=False)
    b2c_d = nc.declare_dram_parameter("b2c", [D, 1], F32, isOutput=False)
    s1c_d = nc.declare_dram_parameter("s1c", [ROWS, 1], F32, isOutput=False)
    s2r_d = nc.declare_dram_parameter("s2r", [B, N2], F32, isOutput=False)
    cm_d = nc.declare_dram_parameter("cm", [ROWS, N2], F32, isOutput=False)

    pred_d = nc.declare_dram_parameter("pred", [ROWS, N2], F32, isOutput=True)
    mask_d = nc.declare_dram_parameter("mask", [ROWS, N2], F32, isOutput=True)
    ycon_d = nc.declare_dram_parameter("ycon", [ROWS, N2], F32, isOutput=True)

    with tile.TileContext(nc) as tc:
        with (
            tc.tile_pool(name="const", bufs=1) as const,
            tc.tile_pool(name="sb", bufs=1) as sb,
            tc.tile_pool(name="hidp", bufs=6) as hidp,
            tc.tile_pool(name="pps", bufs=1, space="PSUM") as pps,
            tc.tile_pool(name="predps", bufs=1, space="PSUM") as predps,
        ):
            # ---- constants ----
            w1a = const.tile([D, D], BF16)
            nc.sync.dma_start(out=w1a[:], in_=w1a_d[:])
            w1b = const.tile([D, D], BF16)
            nc.sync.dma_start(out=w1b[:], in_=w1b_d[:])
            b1c = const.tile([D, 1], F32)
            nc.sync.dma_start(out=b1c[:], in_=b1c_d[:])
            b2c = const.tile([D, 1], F32)
            nc.sync.dma_start(out=b2c[:], in_=b2c_d[:])
            s1c = const.tile([ROWS, 1], F32)
            nc.sync.dma_start(out=s1c[:], in_=s1c_d[:])

            # 32 stationary variants (host-built): variant r has W2 in
            # column r of its [D, 32] block, zeros elsewhere.
            w2g = const.tile([D, 32 * 32], BF16)
            nc.sync.dma_start(out=w2g[:], in_=w2g_d[:])

            # m1 column: S1 values are small non-negative ints; mask = min(S,1)
            m1c = const.tile([ROWS, 1], F32)
            nc.vector.tensor_scalar(
                out=m1c[:], in0=s1c[:], scalar1=1.0, scalar2=None,
                op0=mybir.AluOpType.min,
            )

            # ---- projections: p2T = (W1b)^T-reduce over d of h2T; p1bT ----
            p2sb = []
            p1b = sb.tile([D, ROWS], F32, tag="p1b")
            for b in range(B):
                h2sb = sb.tile([D, N2], BF16, tag=f"h2sb{b}")
                nc.sync.dma_start(out=h2sb[:], in_=h2t_d[b])
                p2ps = pps.tile([D, N2], F32, tag="p2ps")
                for lo, hi in CHUNKS:
                    nc.tensor.matmul(
                        out=p2ps[:, lo:hi], lhsT=w1b[:], rhs=h2sb[:, lo:hi],
                        start=True, stop=True,
                    )
                p2 = sb.tile([D, N2], BF16, tag=f"p2_{b}")
                nc.vector.tensor_copy(out=p2[:], in_=p2ps[:])
                p2sb.append(p2)

                h1sb = sb.tile([D, N1S], BF16, tag=f"h1sb{b}")
                nc.sync.dma_start(out=h1sb[:], in_=h1t_d[b])
                p1ps = pps.tile([D, N1S], F32, tag="p1ps")
                nc.tensor.matmul(
                    out=p1ps[:], lhsT=w1a[:], rhs=h1sb[:], start=True, stop=True
                )
                # p1bT column block for this b, with b1 added (ACT bias)
                nc.scalar.activation(
                    out=p1b[:, b * N1S:(b + 1) * N1S], in_=p1ps[:],
                    func=mybir.ActivationFunctionType.Identity, bias=b1c[:], scale=1.0,
                )

            # ---- mask / y_contact (independent; overlaps main loop) ----
            m2rep = sb.tile([ROWS, N2], F32, tag="m2rep")
            for b in range(B):
                s2b = s2r_d[b]
                bc = bass.AP(
                    tensor=s2b.tensor, offset=s2b.offset,
                    ap=[[0, N1S]] + list(s2b.ap),
                )
                nc.sync.dma_start(out=m2rep[b * N1S:(b + 1) * N1S, :], in_=bc)
            maskt = sb.tile([ROWS, N2], F32, tag="maskt")
            # mask = min(m2,1) * m1col
            nc.vector.tensor_scalar(
                out=maskt[:], in0=m2rep[:], scalar1=1.0, scalar2=m1c[:],
                op0=mybir.AluOpType.min, op1=mybir.AluOpType.mult,
            )
            nc.sync.dma_start(out=mask_d[:], in_=maskt[:])

            cmsb = sb.tile([ROWS, N2], F32, tag="cmsb")
            nc.sync.dma_start(out=cmsb[:], in_=cm_d[:])
            cmb = sb.tile([ROWS, N2], F32, tag="cmb")
            nc.gpsimd.tensor_scalar(
                out=cmb[:], in0=cmsb[:], scalar1=TH, scalar2=None,
                op0=mybir.AluOpType.is_lt,
            )
            yt = sb.tile([ROWS, N2], F32, tag="yt")
            nc.gpsimd.tensor_tensor(
                out=yt[:], in0=cmb[:], in1=maskt[:], op=mybir.AluOpType.mult
            )
            nc.sync.dma_start(out=ycon_d[:], in_=yt[:])

            # ---- main loop ----
            # PE matmul outputs may only start at psum partition 0/32/64, so
            # accumulate into two [64, N2] tiles (rows 0-63 and 64-125).
            pred_ps = [
                predps.tile([64, N2], F32, tag="pred0", name="pred_ps0"),
                predps.tile([64, N2], F32, tag="pred1", name="pred_ps1"),
            ]
            started = set()
            group_last = {(0, 0): 31, (0, 1): 63, (1, 0): 95, (1, 1): ROWS - 1}
            for r in range(ROWS):
                b = r // N1S
                hid = hidp.tile([D, N2], BF16, tag="hid")
                col = p1b[:, r:r + 1]
                if r % 4 == 3:
                    nc.scalar.activation(
                        out=hid[:], in_=p2sb[b][:],
                        func=mybir.ActivationFunctionType.Relu,
                        bias=col, scale=1.0,
                    )
                else:
                    nc.vector.tensor_scalar(
                        out=hid[:], in0=p2sb[b][:], scalar1=col, scalar2=0.0,
                        op0=mybir.AluOpType.add, op1=mybir.AluOpType.max,
                    )
                half, sub = divmod(r, 64)
                g = sub // 32
                lane = r % 32
                lhsT = w2g[:, lane * 32:(lane + 1) * 32]
                for c, (lo, hi) in enumerate(CHUNKS):
                    key = (half, g, c)
                    st = key not in started
                    started.add(key)
                    nc.tensor.matmul(
                        out=pred_ps[half][g * 32:(g + 1) * 32, lo:hi],
                        lhsT=lhsT, rhs=hid[:, lo:hi],
                        start=st, stop=(r == group_last[(half, g)]),
                        skip_group_check=True,
                    )

            predsb = sb.tile([ROWS, N2], F32, tag="predsb")
            nc.scalar.activation(
                out=predsb[0:64, :], in_=pred_ps[0][:],
                func=mybir.ActivationFunctionType.Identity,
                bias=b2c[0:64, :], scale=1.0,
            )
            nc.scalar.activation(
                out=predsb[64:ROWS, :], in_=pred_ps[1][0:ROWS - 64, :],
                func=mybir.ActivationFunctionType.Identity,
                bias=b2c[0:ROWS - 64, :], scale=1.0,
            )
            nc.sync.dma_start(out=pred_d[:], in_=predsb[:])

    _split_waits(nc)
    return nc


def _marshal(inputs):
    """Full inputs -> list of 8 per-core input maps."""
    S1 = np.asarray(inputs["S_mol1"]).astype(np.float32)    # [B, N1]
    S2 = np.asarray(inputs["S_mol2"]).astype(np.float32)    # [B, N2]
    h1 = np.asarray(inputs["h_mol1"], dtype=np.float32)     # [B, N1, D]
    h2 = np.asarray(inputs["h_mol2"], dtype=np.float32)     # [B, N2, D]
    cm = np.asarray(inputs["contact_map"], dtype=np.float32)
    W1 = np.asarray(inputs["W1"], dtype=np.float32)         # [2D, D]
    b1 = np.asarray(inputs["b1"], dtype=np.float32)         # [D]
    W2 = np.asarray(inputs["W2"], dtype=np.float32)         # [D, 1]
    b2 = np.asarray(inputs["b2"], dtype=np.float32)         # [1]

    pad = N1P - N1
    S1p = np.pad(S1, ((0, 0), (0, pad)))
    h1p = np.pad(h1, ((0, 0), (0, pad), (0, 0)))
    cmp_ = np.pad(cm, ((0, 0), (0, pad), (0, 0)))

    h2t = np.ascontiguousarray(h2.transpose(0, 2, 1)).astype(BF16NP)  # [B,D,N2]
    w1a = np.ascontiguousarray(W1[:D]).astype(BF16NP)
    w1b = np.ascontiguousarray(W1[D:]).astype(BF16NP)
    w2g = np.zeros((D, 32, 32), np.float32)
    for r in range(32):
        w2g[:, r, r] = W2[:, 0]
    w2g = np.ascontiguousarray(w2g.reshape(D, 32 * 32)).astype(BF16NP)
    b1c = np.ascontiguousarray(b1.reshape(D, 1))
    b2c = np.full((D, 1), float(b2[0]), np.float32)
    s2r = np.ascontiguousarray(S2)

    in_maps = []
    for c in range(NCORES):
        sl = slice(c * N1S, (c + 1) * N1S)
        h1s = h1p[:, sl, :]                                   # [B, N1S, D]
        h1t = np.ascontiguousarray(h1s.transpose(0, 2, 1)).astype(BF16NP)
        s1c = np.ascontiguousarray(S1p[:, sl].reshape(ROWS, 1))
        cmc = np.ascontiguousarray(cmp_[:, sl, :].reshape(ROWS, N2))
        in_maps.append({
            "h1t": h1t, "h2t": h2t, "w1a": w1a, "w1b": w1b, "w2g": w2g,
            "b1c": b1c, "b2c": b2c, "s1c": s1c, "s2r": s2r, "cm": cmc,
        })
    return in_maps


def _gather(results):
    """Per-core outputs -> full-shape tuple (pred, y, mask)."""
    outs = []
    for name in ("pred", "ycon", "mask"):
        per_core = np.stack([results[c][name] for c in range(NCORES)])
        # [NCORES, ROWS, N2] -> [NCORES, B, N1S, N2] -> [B, NCORES*N1S, N2]
        full = per_core.reshape(NCORES, B, N1S, N2).transpose(1, 0, 2, 3)
        full = full.reshape(B, N1P, N2)[:, :N1, :]
        outs.append(np.ascontiguousarray(full.reshape(B, N1 * N2), dtype=np.float32))
    pred, ycon, mask = outs
    return pred, ycon, mask


_NC_CACHE = None


def get_nc():
    global _NC_CACHE
    if _NC_CACHE is None:
        _NC_CACHE = build_nc()
    return _NC_CACHE


def kernel(**inputs):
    nc = get_nc()
    in_maps = _marshal(inputs)
    res = run_bass_kernel_spmd(nc, in_maps, core_ids=list(range(NCORES)))
    return _gather(res.results)
